# revision 1
# baseline (speedup 1.0000x reference)
"""Trainium2 Bass kernel for nn_KbModel: per-point 100-step Adam inverse-distortion
solve + fisheye reprojection, data-parallel over 8 NeuronCores.

Contract: kernel(**inputs) takes FULL inputs {"inputs": [N,2] f32, "k_vector": [5] f32}
and returns the FULL [N,2] f32 output. Self-contained.
"""
import sys

sys.path.insert(0, "/opt/trn_rl_repo")

import numpy as np

import concourse.bacc as bacc
from concourse import mybir
from concourse.tile import TileContext
from concourse.bass_utils import run_bass_kernel_spmd

AF = mybir.ActivationFunctionType
ALU = mybir.AluOpType
F32 = mybir.dt.float32

# Problem constants (hardcoded per spec)
N_FULL = 4_194_304
N_CORES = 8
N_CORE = N_FULL // N_CORES          # 524288 points per core
P = 128
CHUNKS = 4
F = N_CORE // (CHUNKS * P)          # 1024 columns per chunk
STEPS = 100
LR = 0.01
B1, B2, EPS = 0.9, 0.999, 1e-8
F_X, F_Y = 600.0, 600.0
C_X, C_Y = 512.0, 512.0

_CACHE = {}

import os
K_STEPS = int(os.environ.get("K_STEPS", STEPS))
K_P3 = os.environ.get("K_P3", "gp")
K_G2 = os.environ.get("K_G2", "gp")
K_U = os.environ.get("K_U", "gp")


def _sincos_coeffs():
    # minimax-ish (Chebyshev LS) fits: sin(x)/x and cos(x) as polys in t=x^2,
    # on x in [-1.25, 1.25] (theta after 100 Adam steps stays within ~±1.05)
    from numpy.polynomial import chebyshev as C
    tmax = 1.25 ** 2
    t = np.linspace(0, tmax, 20001)
    x = np.sqrt(t)
    sfit = C.Chebyshev.fit(t, np.sinc(x / np.pi), 5, domain=[0, tmax])
    cfit = C.Chebyshev.fit(t, np.cos(x), 5, domain=[0, tmax])
    sa = sfit.convert(kind=np.polynomial.Polynomial).coef  # sin(x) = x * sum sa_i t^i
    cb = cfit.convert(kind=np.polynomial.Polynomial).coef
    return sa, cb


def _adam_consts():
    """Per-step scale/bias for x = sqrt(VV)/alpha_t + eps*(1-B1^t)/LR."""
    scales, biases = [], []
    for t in range(1, STEPS + 1):
        b1t = 1.0 - B1 ** t
        b2t = 1.0 - B2 ** t
        g_t = np.sqrt(((1.0 - B2) / (1.0 - B1) ** 2) / b2t)  # sqrt(0.1/b2t)
        alpha = LR / (b1t * g_t)
        scales.append(np.float32(1.0 / alpha))
        biases.append(np.float32(EPS * b1t / LR))
    return scales, biases


def _build_program(kv):
    """Build (nc, wm, ctab) for a given k_vector (weights/ctab returned as arrays)."""
    k = kv.astype(np.float64)
    S = (2.0 / N_FULL) * (1.0 - B1)
    # fp-poly coeffs (of theta^1..theta^3), S-folded; cc1 = const term
    cc1 = np.float32(k[1] * S)
    cders = [2.0 * k[2] * S, 3.0 * k[3] * S, 4.0 * k[4] * S]

    # weight matrices [128, 8*128]: blocks I, k1 I..k4 I, c2 I, c3 I, c4 I
    eye = np.eye(P, dtype=np.float64)
    blocks = [eye, k[1] * eye, k[2] * eye, k[3] * eye, k[4] * eye,
              cders[0] * eye, cders[1] * eye, cders[2] * eye]
    wm = np.concatenate(blocks, axis=1).astype(np.float32)  # [128, 1024]

    ad_scales, ad_biases = _adam_consts()
    ctab = np.zeros((P, 128), dtype=np.float32)
    for t in range(STEPS):
        ctab[:, t] = ad_biases[t]
    ctab[:, 100] = np.float32(-C_X / F_X)
    ctab[:, 101] = np.float32(k[0])
    for i in range(5):
        ctab[:, 102 + i] = np.float32(k[i])
    ctab[:, 107] = np.float32(C_X)

    sa, cb = _sincos_coeffs()

    nc = bacc.Bacc("TRN2", target_bir_lowering=False)
    inp = nc.dram_tensor("inp", [N_CORE, 2], F32, kind="ExternalInput")
    wmt = nc.dram_tensor("wm", [P, 8 * P], F32, kind="ExternalInput")
    ctt = nc.dram_tensor("ctab", [P, 128], F32, kind="ExternalInput")
    out = nc.dram_tensor("out", [N_CORE, 2], F32, kind="ExternalOutput")

    inp_r = inp.rearrange("(c p f) t -> c p f t", c=CHUNKS, p=P)
    out_r = out.rearrange("(c p f) t -> c p f t", c=CHUNKS, p=P)

    import contextlib
    with TileContext(nc) as tc, contextlib.ExitStack() as ctx:
        singles = ctx.enter_context(tc.tile_pool(name="singles", bufs=1))
        state = ctx.enter_context(tc.tile_pool(name="state", bufs=1))
        io = ctx.enter_context(tc.tile_pool(name="io", bufs=2))
        tmp = ctx.enter_context(tc.tile_pool(name="tmp", bufs=2))
        fin = ctx.enter_context(tc.tile_pool(name="fin", bufs=1))
        pe_pool = ctx.enter_context(tc.tile_pool(name="pe", bufs=2, space="PSUM"))
        pf_pool = ctx.enter_context(tc.tile_pool(name="pf", bufs=2, space="PSUM"))

        wt = singles.tile([P, 8 * P], F32)
        ct = singles.tile([P, 128], F32)
        nc.sync.dma_start(wt[:], wmt[:])
        nc.sync.dma_start(ct[:], ctt[:])

        W = lambda i: wt[:, i * P:(i + 1) * P]
        bias_mc = ct[:, 100:101]
        bias_k0 = ct[:, 101:102]
        kap = lambda i: ct[:, 102 + i:103 + i]
        bias_cx = ct[:, 107:108]

        th, mm, vv, rk = [], [], [], []
        for c in range(CHUNKS):
            th.append(state.tile([P, F], F32, tag=f"th{c}", name=f"th{c}"))
            mm.append(state.tile([P, F], F32, tag=f"m{c}", name=f"m{c}"))
            vv.append(state.tile([P, F], F32, tag=f"vv{c}", name=f"vv{c}"))
            rk.append(state.tile([P, F], F32, tag=f"rk{c}", name=f"rk{c}"))

        # ---- pre-loop: load inputs, compute rk0 = k0 - ru, zero state ----
        for c in range(CHUNKS):
            tin = io.tile([P, F, 2], F32, tag="tin")
            nc.sync.dma_start(tin[:], inp_r[c])
            mx = tmp.tile([P, F], F32, tag="p2")
            my = tmp.tile([P, F], F32, tag="p3")
            nc.scalar.activation(mx[:], tin[:, :, 0], AF.Identity, bias=bias_mc, scale=1.0 / F_X)
            nc.scalar.activation(my[:], tin[:, :, 1], AF.Identity, bias=bias_mc, scale=1.0 / F_Y)
            mx2 = tmp.tile([P, F], F32, tag="G")
            my2 = tmp.tile([P, F], F32, tag="sV")
            nc.scalar.activation(mx2[:], mx[:], AF.Square)
            nc.scalar.activation(my2[:], my[:], AF.Square)
            nc.vector.tensor_add(mx2[:], mx2[:], my2[:])       # ru2
            ru = tmp.tile([P, F], F32, tag="rec")
            nc.scalar.activation(ru[:], mx2[:], AF.Sqrt)
            nc.scalar.activation(rk[c][:], ru[:], AF.Identity, bias=bias_k0, scale=-1.0)
            nc.gpsimd.memset(th[c][:], 0.0)
            nc.gpsimd.memset(mm[c][:], 0.0)
            nc.gpsimd.memset(vv[c][:], 0.0)

        ad_scales, _ = _adam_consts()
        NSUB = F // 512

        # ---- 100 Adam steps ----
        for t in range(1, K_STEPS + 1):
            for c in range(CHUNKS):
                p2 = tmp.tile([P, F], F32, tag="p2")
                nc.scalar.activation(p2[:], th[c][:], AF.Square)
                p3 = tmp.tile([P, F], F32, tag="p3")
                (nc.gpsimd if K_P3 == 'gp' else nc.vector).tensor_mul(p3[:], p2[:], th[c][:])
                p4 = tmp.tile([P, F], F32, tag="p4esb")
                nc.scalar.activation(p4[:], p2[:], AF.Square)

                pe = pe_pool.tile([P, F], F32, tag="pe")
                e_rhs = [(0, rk[c]), (1, th[c]), (2, p2), (3, p3), (4, p4)]
                for wi, (widx, rhs) in enumerate(e_rhs):
                    for s in range(NSUB):
                        nc.tensor.matmul(pe[:, s * 512:(s + 1) * 512], W(widx),
                                         rhs[:, s * 512:(s + 1) * 512],
                                         start=(wi == 0), stop=(wi == len(e_rhs) - 1))
                pf = pf_pool.tile([P, F], F32, tag="pf")
                f_rhs = [(5, th[c]), (6, p2), (7, p3)]
                for wi, (widx, rhs) in enumerate(f_rhs):
                    for s in range(NSUB):
                        nc.tensor.matmul(pf[:, s * 512:(s + 1) * 512], W(widx),
                                         rhs[:, s * 512:(s + 1) * 512],
                                         start=(wi == 0), stop=(wi == len(f_rhs) - 1))

                esb = tmp.tile([P, F], F32, tag="p4esb")
                nc.scalar.activation(esb[:], pe[:], AF.Copy)
                G = tmp.tile([P, F], F32, tag="G")
                # G = (pf + cc1) * e
                nc.vector.scalar_tensor_tensor(G[:], pf[:], float(cc1), esb[:], ALU.add, ALU.mult)
                # m = m*B1 + G
                nc.vector.scalar_tensor_tensor(mm[c][:], mm[c][:], B1, G[:], ALU.mult, ALU.add)
                # G <- G^2 ; VV = VV*B2 + G^2
                (nc.gpsimd.tensor_mul(G[:], G[:], G[:]) if K_G2 == 'gp'
                 else nc.scalar.activation(G[:], G[:], AF.Square))
                nc.vector.scalar_tensor_tensor(vv[c][:], vv[c][:], B2, G[:], ALU.mult, ALU.add)
                # x = sqrt(VV)*(1/alpha_t) + eps_t  -> rec = 1/x -> u = m*rec -> th -= u
                sv = tmp.tile([P, F], F32, tag="sV")
                nc.scalar.activation(sv[:], vv[c][:], AF.Sqrt)
                nc.scalar.activation(sv[:], sv[:], AF.Identity, bias=ct[:, t - 1:t],
                                     scale=float(ad_scales[t - 1]))
                rec = tmp.tile([P, F], F32, tag="rec")
                nc.vector.reciprocal_approx_fast(out=rec[:], in_=sv[:])
                (nc.gpsimd if K_U == 'gp' else nc.vector).tensor_mul(rec[:], mm[c][:], rec[:])
                nc.vector.scalar_tensor_tensor(th[c][:], rec[:], -1.0, th[c][:], ALU.mult, ALU.add)

        # ---- epilogue: reprojection ----
        sa, cb = _sincos_coeffs()
        for c in range(CHUNKS):
            tin = io.tile([P, F, 2], F32, tag="tin")
            nc.sync.dma_start(tin[:], inp_r[c])
            mx = fin.tile([P, F], F32, tag="fa")
            my = fin.tile([P, F], F32, tag="fb")
            nc.scalar.activation(mx[:], tin[:, :, 0], AF.Identity, bias=bias_mc, scale=1.0 / F_X)
            nc.scalar.activation(my[:], tin[:, :, 1], AF.Identity, bias=bias_mc, scale=1.0 / F_Y)
            q1 = tmp.tile([P, F], F32, tag="G")
            q2 = tmp.tile([P, F], F32, tag="sV")
            nc.scalar.activation(q1[:], mx[:], AF.Square)
            nc.scalar.activation(q2[:], my[:], AF.Square)
            nc.vector.tensor_add(q1[:], q1[:], q2[:])          # ru2
            ru = tmp.tile([P, F], F32, tag="rec")
            nc.scalar.activation(ru[:], q1[:], AF.Sqrt)
            inv = fin.tile([P, F], F32, tag="ff")
            nc.vector.reciprocal_approx_accurate(out=inv[:], in_=ru[:], scratch=q2[:])
            # theta powers
            t2 = fin.tile([P, F], F32, tag="fg")
            nc.scalar.activation(t2[:], th[c][:], AF.Square)
            t4 = fin.tile([P, F], F32, tag="fh")
            nc.scalar.activation(t4[:], t2[:], AF.Square)
            t6 = tmp.tile([P, F], F32, tag="p2")
            nc.vector.tensor_mul(t6[:], t2[:], t4[:])
            t8 = tmp.tile([P, F], F32, tag="p3")
            nc.scalar.activation(t8[:], t4[:], AF.Square)
            t10 = tmp.tile([P, F], F32, tag="p4esb")
            nc.vector.tensor_mul(t10[:], t4[:], t6[:])
            # s = th * (sa0 + sa1 t2 + ... + sa5 t10)
            sacc = fin.tile([P, F], F32, tag="fl")
            nc.vector.tensor_scalar(sacc[:], t2[:], float(sa[1]), float(sa[0]), ALU.mult, ALU.add)
            nc.vector.scalar_tensor_tensor(sacc[:], t4[:], float(sa[2]), sacc[:], ALU.mult, ALU.add)
            nc.vector.scalar_tensor_tensor(sacc[:], t6[:], float(sa[3]), sacc[:], ALU.mult, ALU.add)
            nc.vector.scalar_tensor_tensor(sacc[:], t8[:], float(sa[4]), sacc[:], ALU.mult, ALU.add)
            nc.vector.scalar_tensor_tensor(sacc[:], t10[:], float(sa[5]), sacc[:], ALU.mult, ALU.add)
            nc.vector.tensor_mul(sacc[:], sacc[:], th[c][:])   # sin(theta)
            # NOTE: cos(theta) is NOT needed: th = arctan2(|sin|, cos) == |theta|
            # exactly for |theta| < pi/2, and d(th) uses |theta| powers directly.
            # d(|th|) = k0 + k1|th| + k2 t2 + k3 |th|^3 + k4 t4
            tha = tmp.tile([P, F], F32, tag="G")
            nc.scalar.activation(tha[:], th[c][:], AF.Abs)
            th3 = tmp.tile([P, F], F32, tag="sV")
            nc.vector.tensor_mul(th3[:], tha[:], t2[:])
            d = fin.tile([P, F], F32, tag="fp")
            nc.vector.tensor_scalar(d[:], tha[:], kap(1), kap(0), ALU.mult, ALU.add)
            nc.vector.scalar_tensor_tensor(d[:], t2[:], kap(2), d[:], ALU.mult, ALU.add)
            nc.vector.scalar_tensor_tensor(d[:], th3[:], kap(3), d[:], ALU.mult, ALU.add)
            nc.vector.scalar_tensor_tensor(d[:], t4[:], kap(4), d[:], ALU.mult, ALU.add)
            # px = s*mx/ru etc; u = d*px*600 + 512
            nc.vector.tensor_mul(mx[:], mx[:], inv[:])
            nc.vector.tensor_mul(my[:], my[:], inv[:])
            nc.vector.tensor_mul(mx[:], mx[:], sacc[:])        # px
            nc.vector.tensor_mul(my[:], my[:], sacc[:])        # py
            nc.vector.tensor_mul(mx[:], mx[:], d[:])
            nc.vector.tensor_mul(my[:], my[:], d[:])
            tout = io.tile([P, F, 2], F32, tag="tout")
            nc.scalar.activation(tout[:, :, 0], mx[:], AF.Identity, bias=bias_cx, scale=F_X)
            nc.scalar.activation(tout[:, :, 1], my[:], AF.Identity, bias=bias_cx, scale=F_Y)
            nc.sync.dma_start(out_r[c], tout[:])

    nc.compile()
    return nc, wm, ctab


def kernel(inputs: np.ndarray, k_vector: np.ndarray) -> np.ndarray:
    inputs = np.ascontiguousarray(inputs, dtype=np.float32)
    k_vector = np.ascontiguousarray(k_vector, dtype=np.float32)
    key = k_vector.tobytes()
    if key not in _CACHE:
        _CACHE[key] = _build_program(k_vector)
    nc, wm, ctab = _CACHE[key]
    in_maps = []
    for i in range(N_CORES):
        shard = np.ascontiguousarray(inputs[i * N_CORE:(i + 1) * N_CORE])
        in_maps.append({"inp": shard, "wm": wm, "ctab": ctab})
    res = run_bass_kernel_spmd(nc, in_maps, core_ids=list(range(N_CORES)))
    kernel._LAST_RESULTS = res
    return np.concatenate([r["out"] for r in res.results], axis=0)


if __name__ == "__main__":
    rng = np.random.default_rng(0)
    inputs = (rng.random((N_FULL, 2), dtype=np.float32) * 1024.0)
    kv = np.array([1.0, -0.01, 0.005, -0.002, 0.0005], dtype=np.float32)
    out = kernel(inputs, kv)
    print(out.shape, out.dtype, out[:2])

